# revision 1
# baseline (speedup 1.0000x reference)
"""Trainium2 Bass kernel for the dense transformer block (nn_Block_44873818308807).

B=16, S=1024, E=768, H=12, D=64, F=3072. Data-parallel over batch across the
8 NeuronCores (2 batch elements per core, no collectives). Per core:
  - LN1 token-major -> PE-transpose to feature-major hT
  - QKV in bf16 (q/k feature-major, v token-major per head with a ones column
    appended so the PV matmul also produces softmax denominators)
  - causal attention per (head-pair, 512-query strip): scoresT [t,s] on PE,
    exp on the Scalar engine (batched over tau-pairs, no max-subtraction --
    LN-bounded scores make exp safe), block-causal structure with an
    upper-triangular mask multiply on diagonal blocks; PV with v stationary
    -> unnormalized oT [65, s] with the sums row; softmax normalization via
    PE-transpose of sums -> reciprocal in token-major -> transpose back ->
    PE outer-product broadcast -> one multiply per head-pair chunk
  - proj + residual in fp32r, LN2 fused into the eviction -> h2T
  - FFN in fp32r: W1 resident per batch, FFN1 weight-stationary -> gelu
    (erf-based, matches the reference) -> FFN2 with W2 streamed per 128-row
    chunk, four 512-token sigma accumulators in PSUM; LNf + residual fused
    into the eviction.
Matmul dtypes: attention+QKV in bf16 (clock-robust 1 cycle/column), everything
else float32r. Measured end-to-end relative error ~7e-4 vs the fp32 reference.
"""
import sys
for _p in ("/opt/trn_rl_repo",):
    if _p not in sys.path:
        sys.path.insert(0, _p)
import numpy as np
import ml_dtypes
import concourse.bass as bass
import concourse.mybir as mybir
import concourse.tile as tile
from concourse import bacc
from concourse.bass_utils import run_bass_kernel_spmd
from concourse.masks import make_identity



f32 = mybir.dt.float32
f32r = mybir.dt.float32r
bf16 = mybir.dt.bfloat16
AF = mybir.ActivationFunctionType
ALU = mybir.AluOpType

B_LOC, S, E, H, D, F = 2, 1024, 768, 12, 64, 3072
EC = E // 128   # 6 feature chunks
FC = F // 128   # 24 ffn chunks
NSIG = S // 128  # 8 token chunks per batch
EPS = 1e-5


def declare_io(nc, b_loc=B_LOC):
    t = {}
    t["x"] = nc.dram_tensor("x", [b_loc, S, E], f32, kind="ExternalInput").ap()
    for w in ["wq", "wk", "wv"]:
        t[w] = nc.dram_tensor(w, [E, E], mybir.dt.bfloat16,
                              kind="ExternalInput").ap()
    t["wp"] = nc.dram_tensor("wp", [E, E], f32r, kind="ExternalInput").ap()
    t["w1"] = nc.dram_tensor("w1", [E, F], f32r, kind="ExternalInput").ap()
    t["w2"] = nc.dram_tensor("w2", [F, E], f32r, kind="ExternalInput").ap()
    for v in ["bq", "bk", "bv", "bp", "b2"]:
        t[v] = nc.dram_tensor(v, [E], f32, kind="ExternalInput").ap()
    t["b1"] = nc.dram_tensor("b1", [F], f32, kind="ExternalInput").ap()
    for v in ["ln1_g", "ln1_b", "ln2_g", "ln2_b", "lnf_g", "lnf_b"]:
        t[v] = nc.dram_tensor(v, [E], f32, kind="ExternalInput").ap()
    t["tri"] = nc.dram_tensor("tri", [128, 128], f32, kind="ExternalInput").ap()
    t["sel2"] = nc.dram_tensor("sel2", [2, 128], f32r, kind="ExternalInput").ap()
    t["out"] = nc.dram_tensor("out", [b_loc, S, E], f32, kind="ExternalOutput").ap()
    t["x2_buf"] = nc.dram_tensor("x2_buf", [b_loc, S, E], f32).ap()
    return t


def layernorm_tok(nc, pool, out_ap, in_ap, eps_t, zero_bias, g_bc=None, b_bc=None):
    """out = LN(in) token-major [128, 768]. in_ap may be PSUM or SBUF."""
    stats = pool.tile([128, 3, 6], f32, tag="ln_stats")
    xr = in_ap.rearrange("p (g d) -> p g d", d=256)
    for g in range(3):
        nc.vector.bn_stats(out=stats[:, g, :], in_=xr[:, g, :])
    mv = pool.tile([128, 2], f32, tag="ln_mv")
    nc.vector.bn_aggr(out=mv, in_=stats)
    rstd = pool.tile([128, 1], f32, tag="ln_rstd")
    nc.scalar.activation(out=rstd, in_=mv[:, 1:2], func=AF.Sqrt, bias=eps_t,
                         scale=1.0)
    nc.vector.reciprocal(rstd, rstd)
    nc.vector.tensor_scalar(out=out_ap, in0=in_ap, scalar1=mv[:, 0:1],
                            scalar2=rstd, op0=ALU.subtract, op1=ALU.mult)
    if g_bc is not None:
        nc.vector.tensor_mul(out_ap, out_ap, g_bc)
    if b_bc is not None:
        nc.vector.tensor_add(out_ap, out_ap, b_bc)


def transpose_768(nc, pspool, dst, sigma, src, ident):
    """src [128 tokens, 768] -> dst[:, c, sigma*128:+128] for c in 0..5 via PE."""
    ps4 = pspool.tile([128, 512], f32, tag="t4")
    ps2 = pspool.tile([128, 256], f32, tag="t2")
    for c in range(4):
        nc.tensor.matmul(ps4[:, c * 128:(c + 1) * 128], lhsT=src[:, c * 128:(c + 1) * 128],
                         rhs=ident, is_transpose=True, start=(c == 0), stop=(c == 3))
    for c in range(2):
        nc.tensor.matmul(ps2[:, c * 128:(c + 1) * 128], lhsT=src[:, (c + 4) * 128:(c + 5) * 128],
                         rhs=ident, is_transpose=True, start=(c == 0), stop=(c == 1))
    sl = slice(sigma * 128, (sigma + 1) * 128)
    nc.vector.tensor_copy(dst[:, 0:4, sl], ps4.rearrange("p (c n) -> p c n", n=128))
    nc.vector.tensor_copy(dst[:, 4:6, sl], ps2.rearrange("p (c n) -> p c n", n=128))


def build(nc, tc, t, cfg):
    """Emit the whole per-core kernel under TileContext tc."""
    b_loc = cfg.get("b_loc", B_LOC)
    ln1_aff = cfg.get("ln1_aff", False)   # include gamma/beta ops
    ln2_aff = cfg.get("ln2_aff", False)
    lnf_aff = cfg.get("lnf_aff", False)
    add_bp = cfg.get("add_bp", False)
    add_b2 = cfg.get("add_b2", False)

    consts_cm = tc.tile_pool(name="consts", bufs=1)
    consts = consts_cm.__enter__()
    ident = consts.tile([128, 128], f32)
    make_identity(nc, ident)
    tri_t = consts.tile([128, 128], f32)
    nc.sync.dma_start(out=tri_t, in_=t["tri"])
    sel2_t = consts.tile([2, 128], f32r)
    nc.sync.dma_start(out=sel2_t, in_=t["sel2"])
    warm_t = consts.tile([128, 128], f32r)
    nc.vector.tensor_copy(warm_t, ident)
    eps_t = consts.tile([128, 1], f32)
    nc.vector.memset(eps_t, EPS)
    zero_t = consts.tile([128, 1], f32)
    nc.vector.memset(zero_t, 0.0)
    # per-partition bias tiles for feature-major adds: [128, EC]
    bq_sb = consts.tile([128, EC], f32)
    bk_sb = consts.tile([128, EC], f32)
    nc.sync.dma_start(out=bq_sb, in_=t["bq"].rearrange("(c p) -> p c", p=128))
    nc.sync.dma_start(out=bk_sb, in_=t["bk"].rearrange("(c p) -> p c", p=128))
    b1_sb = consts.tile([128, FC], f32)
    nc.sync.dma_start(out=b1_sb, in_=t["b1"].rearrange("(c p) -> p c", p=128))
    # broadcast [128, 768] tiles for free-dim adds (token-major)
    bv_bc = consts.tile([128, E], f32)
    nc.gpsimd.dma_start(out=bv_bc, in_=t["bv"].rearrange("(a e) -> a e", a=1).to_broadcast((128, E)))
    bp_bc = b2_bc = None
    if add_bp:
        bp_bc = consts.tile([128, E], f32)
        nc.gpsimd.dma_start(out=bp_bc, in_=t["bp"].rearrange("(a e) -> a e", a=1).to_broadcast((128, E)))
    if add_b2:
        b2_bc = consts.tile([128, E], f32)
        nc.gpsimd.dma_start(out=b2_bc, in_=t["b2"].rearrange("(a e) -> a e", a=1).to_broadcast((128, E)))
    affs = {}
    for nm, flag in [("ln1", ln1_aff), ("ln2", ln2_aff), ("lnf", lnf_aff)]:
        if flag:
            gt = consts.tile([128, E], f32)
            bt_ = consts.tile([128, E], f32)
            nc.gpsimd.dma_start(out=gt, in_=t[nm + "_g"].rearrange("(a e) -> a e", a=1).to_broadcast((128, E)))
            nc.gpsimd.dma_start(out=bt_, in_=t[nm + "_b"].rearrange("(a e) -> a e", a=1).to_broadcast((128, E)))
            affs[nm] = (gt, bt_)
        else:
            affs[nm] = (None, None)

    wq_r = t["wq"].rearrange("(c p) f -> p c f", p=128)
    wk_r = t["wk"].rearrange("(c p) f -> p c f", p=128)
    wv_r = t["wv"].rearrange("(c p) f -> p c f", p=128)
    wp_r = t["wp"].rearrange("(c p) f -> p c f", p=128)
    w1_r = t["w1"].rearrange("(c p) f -> p c f", p=128)

    for b in range(b_loc):
        late_cm = tc.tile_pool(name=f"late{b}", bufs=1)
        late = late_cm.__enter__()
        h2T = late.tile([128, EC, S], f32r, tag="h2T")
        span_cm = tc.tile_pool(name=f"span{b}", bufs=4)
        span = span_cm.__enter__()
        hT = span.tile([128, EC, S], bf16, tag="a25")
        qT = span.tile([128, EC, S], bf16, tag="a25")
        kT = span.tile([128, EC, S], bf16, tag="a25")
        v_all = span.tile([128, NSIG, H, D + 1], bf16, tag="a25")

        # ---------- Phases 1+2: LN1 -> hT, then QKV ----------
        # QKV weight DMAs issue first so they overlap the LN1 compute.
        with nc.named_scope(f"qkv_b{b}"):
            with (
                tc.tile_pool(name=f"p2_{b}", bufs=1) as p2,
                tc.tile_pool(name=f"ps2_{b}", bufs=2, space="PSUM") as ps2,
            ):
                with nc.named_scope(f"warm_b{b}"):
                    psw = ps2.tile([128, 512], f32, tag="q", bufs=2, name="psw")
                    for _w in range(40):
                        nc.tensor.matmul(psw[:, 0:128], lhsT=warm_t,
                                         rhs=warm_t, start=True, stop=True)
                wq_sb = p2.tile([128, EC, E], bf16, tag="w18", bufs=2)
                nc.sync.dma_start(out=wq_sb, in_=wq_r)
                wk_sb = p2.tile([128, EC, E], bf16, tag="w18", bufs=2)
                nc.sync.dma_start(out=wk_sb, in_=wk_r)
                wv_sb = p2.tile([128, EC, E], bf16, tag="w18", bufs=2)
                nc.sync.dma_start(out=wv_sb, in_=wv_r)

                with nc.named_scope(f"ln1_b{b}"):
                    with (
                        tc.tile_pool(name=f"p1_{b}", bufs=2) as p1,
                        tc.tile_pool(name=f"ps1_{b}", bufs=2, space="PSUM") as ps1,
                    ):
                        g1, bb1 = affs["ln1"]
                        for sg in range(NSIG):
                            x_t = p1.tile([128, E], f32, tag="x")
                            nc.sync.dma_start(
                                out=x_t, in_=t["x"][b, sg * 128:(sg + 1) * 128, :])
                            h_t = p1.tile([128, E], f32, tag="h")
                            layernorm_tok(nc, p1, h_t, x_t, eps_t, zero_t, g1, bb1)
                            transpose_768(nc, ps1, hT, sg, h_t, ident)

                for w_sb, dst, bias in [(wq_sb, qT, bq_sb), (wk_sb, kT, bk_sb)]:
                    for fc in range(EC):
                        for j in range(2):
                            psq = ps2.tile([128, 512], f32, tag="q")
                            for e in range(EC):
                                nc.tensor.matmul(
                                    psq, lhsT=w_sb[:, e, fc * 128:(fc + 1) * 128],
                                    rhs=hT[:, e, j * 512:(j + 1) * 512],
                                    start=(e == 0), stop=(e == EC - 1))
                            nc.vector.tensor_scalar_add(
                                out=dst[:, fc, j * 512:(j + 1) * 512], in0=psq,
                                scalar1=bias[:, fc:fc + 1])
                # v token-major per-head (+ones column)
                for _tau in range(NSIG):
                    nc.gpsimd.memset(v_all[:, _tau, :, D:D + 1], 1.0)
                for tau in range(NSIG):
                    psv = ps2.tile([128, E], f32, tag="v", bufs=1)
                    for e in range(EC):
                        nc.tensor.matmul(psv[:, 0:512],
                                         lhsT=hT[:, e, tau * 128:(tau + 1) * 128],
                                         rhs=wv_sb[:, e, 0:512],
                                         start=(e == 0), stop=(e == EC - 1))
                        nc.tensor.matmul(psv[:, 512:768],
                                         lhsT=hT[:, e, tau * 128:(tau + 1) * 128],
                                         rhs=wv_sb[:, e, 512:768],
                                         start=(e == 0), stop=(e == EC - 1))
                    nc.vector.tensor_add(
                        out=v_all[:, tau, 0:8, 0:D],
                        in0=psv[:, 0:512].rearrange("p (h d) -> p h d", d=D),
                        in1=bv_bc[:, 0:512].rearrange("p (h d) -> p h d", d=D))
                    nc.vector.tensor_add(
                        out=v_all[:, tau, 8:12, 0:D],
                        in0=psv[:, 512:768].rearrange("p (h d) -> p h d", d=D),
                        in1=bv_bc[:, 512:768].rearrange("p (h d) -> p h d", d=D))

        # hT dead; oT reuses its slot
        oT = span.tile([128, EC, S], f32r, tag="a25")

        # ---------- Phase 3: attention ----------
        with nc.named_scope(f"attn_b{b}"):
            with (
                tc.tile_pool(name=f"p3_{b}", bufs=1) as p3,
                tc.tile_pool(name=f"ps3_{b}", bufs=1, space="PSUM") as ps3,
            ):
                sums = p3.tile([12, S], f32, tag="sums")
                # heads processed in row-packed pairs: even head on PE rows
                # 0-63, odd head on rows 64-127 -- the two half-array score
                # matmuls run concurrently (distinct row groups), keeping the
                # array fully active so HAM holds the 2.4 GHz clock.
                for hc in range(EC):
                    for j in range(2):
                        js = slice(j * 512, (j + 1) * 512)
                        pTs = [p3.tile([128, NSIG, 512], bf16, tag="pT", bufs=2,
                                       name=f"pT{hh}") for hh in range(2)]
                        psos = [ps3.tile([128, 512], f32, tag="po", bufs=2,
                                         name=f"pso{hh}") for hh in range(2)]
                        ntau = 4 * j + 4
                        n0s = [max(0, (tau - 4 * j)) * 128 for tau in range(ntau)]
                        npairs = ntau // 2

                        def emit_scores_exp(pi):
                            ta, tb = 2 * pi, 2 * pi + 1
                            n0min = n0s[ta]
                            psc2s = [ps3.tile([128, 2, 512], f32, tag="sc2",
                                              bufs=3, name=f"psc2_{hh}")
                                     for hh in range(2)]
                            for i, tau in ((0, ta), (1, tb)):
                                for hh in range(2):
                                    hb = hh * 64
                                    nc.tensor.matmul(
                                        psc2s[hh][:, i, n0min:512],
                                        lhsT=kT[hb:hb + 64, hc,
                                                tau * 128:(tau + 1) * 128],
                                        rhs=qT[hb:hb + 64, hc,
                                               j * 512 + n0min:(j + 1) * 512],
                                        start=True, stop=True)
                            for hh in range(2):
                                nc.scalar.activation(
                                    out=pTs[hh][:, ta:tb + 1, n0min:512],
                                    in_=psc2s[hh][:, :, n0min:512], func=AF.Exp,
                                    bias=zero_t, scale=0.125)
                            for tau in (ta, tb):
                                if tau >= 4 * j:
                                    n0 = n0s[tau]
                                    for hh in range(2):
                                        nc.vector.tensor_mul(
                                            pTs[hh][:, tau, n0:n0 + 128],
                                            pTs[hh][:, tau, n0:n0 + 128], tri_t)
                        def emit_pv(pi):
                            ta, tb = 2 * pi, 2 * pi + 1
                            for tau in (ta, tb):
                                n0 = n0s[tau]
                                for hh in range(2):
                                    nc.tensor.matmul(
                                        psos[hh][0:65, n0:512],
                                        lhsT=v_all[:, tau, 2 * hc + hh, :],
                                        rhs=pTs[hh][:, tau, n0:512],
                                        start=(tau == 0),
                                        stop=(tau == ntau - 1))

                        # software pipeline: PV of pair pi issues after the
                        # scores of pair pi+1, hiding the ACT exp latency
                        emit_scores_exp(0)
                        for pi in range(1, npairs):
                            emit_scores_exp(pi)
                            emit_pv(pi - 1)
                        emit_pv(npairs - 1)
                        # psum -> partition-aligned scratch (DVE), then
                        # partition-shifting placements via SBUF->SBUF DMA
                        for hh in range(2):
                            h = 2 * hc + hh
                            o_scr = p3.tile([65, 512], f32, tag="o_scr", bufs=2)
                            nc.vector.tensor_copy(o_scr, psos[hh][0:65, 0:512])
                            nc.sync.dma_start(out=oT[hh * 64:hh * 64 + 64, hc, js],
                                              in_=o_scr[0:64, :].bitcast(f32r))
                            nc.sync.dma_start(out=sums[h:h + 1, js],
                                              in_=o_scr[64:65, :])
                # normalization dance: transpose sums -> recip (token-major)
                # -> transpose back -> PE outer-product broadcast per head-pair
                pst = ps3.tile([128, 96], f32, tag="sc2", bufs=3)
                for g in range(8):
                    nc.tensor.matmul(pst[:, g * 12:(g + 1) * 12],
                                     lhsT=sums[:, g * 128:(g + 1) * 128],
                                     rhs=ident[0:12, 0:12], is_transpose=True,
                                     start=(g == 0), stop=(g == 7))
                rT = p3.tile([128, 96], f32, tag="rT")
                nc.vector.reciprocal(rT, pst)
                psr_a = ps3.tile([12, 512], f32, tag="po", bufs=2)
                psr_b = ps3.tile([12, 512], f32, tag="po", bufs=2)
                for g in range(8):
                    dstp = psr_a if g < 4 else psr_b
                    nc.tensor.matmul(dstp[:, (g % 4) * 128:(g % 4 + 1) * 128],
                                     lhsT=rT[:, g * 12:(g + 1) * 12],
                                     rhs=ident, is_transpose=True,
                                     start=(g % 4 == 0), stop=(g % 4 == 3))
                r_row = p3.tile([12, S], f32r, tag="r_row")
                nc.vector.tensor_copy(r_row[:, 0:512], psr_a)
                nc.vector.tensor_copy(r_row[:, 512:1024], psr_b)
                for hc in range(EC):
                    r2 = p3.tile([2, S], f32r, tag="r2", bufs=2)
                    nc.sync.dma_start(out=r2[0:1, :], in_=r_row[2 * hc:2 * hc + 1, :])
                    nc.sync.dma_start(out=r2[1:2, :],
                                      in_=r_row[2 * hc + 1:2 * hc + 2, :])
                    for half in range(2):
                        hs = slice(half * 512, (half + 1) * 512)
                        rbc = ps3.tile([128, 512], f32, tag="sc2", bufs=3)
                        nc.tensor.matmul(rbc, lhsT=sel2_t, rhs=r2[:, hs],
                                         start=True, stop=True)
                        nc.vector.tensor_mul(oT[:, hc, hs], oT[:, hc, hs], rbc)

        # ---------- Phase 4: proj + residual + LN2 + h2T ----------
        with nc.named_scope(f"proj_b{b}"):
            with (
                tc.tile_pool(name=f"p4_{b}", bufs=2) as p4,
                tc.tile_pool(name=f"ps4_{b}", bufs=2, space="PSUM") as ps4,
            ):
                wp_sb = p4.tile([128, EC, E], f32r, tag="wp", bufs=1)
                nc.sync.dma_start(out=wp_sb, in_=wp_r)
                g2, bb2 = affs["ln2"]
                for sg in range(NSIG):
                    psp = ps4.tile([128, E], f32, tag="pp")
                    for e in range(EC):
                        nc.tensor.matmul(psp[:, 0:512],
                                         lhsT=oT[:, e, sg * 128:(sg + 1) * 128],
                                         rhs=wp_sb[:, e, 0:512],
                                         start=(e == 0), stop=(e == EC - 1))
                        nc.tensor.matmul(psp[:, 512:768],
                                         lhsT=oT[:, e, sg * 128:(sg + 1) * 128],
                                         rhs=wp_sb[:, e, 512:768],
                                         start=(e == 0), stop=(e == EC - 1))
                    x_t = p4.tile([128, E], f32, tag="x")
                    nc.sync.dma_start(out=x_t, in_=t["x"][b, sg * 128:(sg + 1) * 128, :])
                    x2_t = p4.tile([128, E], f32, tag="x2")
                    nc.vector.tensor_add(x2_t, psp, x_t)
                    if add_bp:
                        nc.vector.tensor_add(x2_t, x2_t, bp_bc)
                    nc.sync.dma_start(out=t["x2_buf"][b, sg * 128:(sg + 1) * 128, :],
                                      in_=x2_t)
                    h2_t = p4.tile([128, E], f32, tag="h2")
                    layernorm_tok(nc, p4, h2_t, x2_t, eps_t, zero_t, g2, bb2)
                    transpose_768(nc, ps4, h2T, sg, h2_t, ident)

        span_cm.__exit__(None, None, None)

        # ---------- Phase 5: FFN + LNf + residual ----------
        with nc.named_scope(f"ffn_b{b}"):
            with tc.tile_pool(name=f"p5_{b}", bufs=1) as p5:
                gf, bbf = affs["lnf"]
                # W1 resident for the whole FFN(b); per-phi slice DMAs so
                # strip 0's phi-loop races the stream instead of waiting.
                w1_full = p5.tile([128, EC, F], f32r, tag="w1f")
                for phi in range(FC):
                    nc.sync.dma_start(
                        out=w1_full[:, :, phi * 128:(phi + 1) * 128],
                        in_=w1_r[:, :, phi * 128:(phi + 1) * 128])
                for j in range(2):
                    js = slice(j * 512, (j + 1) * 512)
                    g_all = p5.tile([128, FC, 512], f32r, tag="g_all")
                    with tc.tile_pool(name=f"ps5a_{b}_{j}", bufs=2, space="PSUM") as ps5a:
                        for phi in range(FC):
                            psf1 = ps5a.tile([128, 512], f32, tag="f1")
                            for e in range(EC):
                                nc.tensor.matmul(
                                    psf1,
                                    lhsT=w1_full[:, e, phi * 128:(phi + 1) * 128],
                                    rhs=h2T[:, e, js],
                                    start=(e == 0), stop=(e == EC - 1))
                            nc.scalar.activation(out=g_all[:, phi, :], in_=psf1,
                                                 func=AF.Gelu,
                                                 bias=b1_sb[:, phi:phi + 1], scale=1.0)
                    with tc.tile_pool(name=f"ps5b_{b}_{j}", bufs=1, space="PSUM") as ps5b:
                        psf2 = [ps5b.tile([128, E], f32, tag=f"f2_{s_}", name=f"psf2_{s_}")
                                for s_ in range(4)]
                        for phi in range(FC):
                            w2_t = p5.tile([128, E], f32r, tag="w2s", bufs=4)
                            nc.sync.dma_start(out=w2_t,
                                              in_=t["w2"][phi * 128:(phi + 1) * 128, :])
                            for s_ in range(4):
                                nc.tensor.matmul(
                                    psf2[s_][:, 0:512],
                                    lhsT=g_all[:, phi, s_ * 128:(s_ + 1) * 128],
                                    rhs=w2_t[:, 0:512],
                                    start=(phi == 0), stop=(phi == FC - 1))
                                nc.tensor.matmul(
                                    psf2[s_][:, 512:768],
                                    lhsT=g_all[:, phi, s_ * 128:(s_ + 1) * 128],
                                    rhs=w2_t[:, 512:768],
                                    start=(phi == 0), stop=(phi == FC - 1))
                        for s_ in range(4):
                            sg = j * 4 + s_
                            if add_b2:
                                f_in = p5.tile([128, E], f32, tag="f_t", bufs=2,
                                               name="f_in")
                                nc.vector.tensor_add(f_in, psf2[s_], b2_bc)
                            else:
                                f_in = psf2[s_]
                            fn_t = p5.tile([128, E], f32, tag="fn_t", bufs=2)
                            layernorm_tok(nc, p5, fn_t, f_in, eps_t, zero_t, gf, bbf)
                            x2r_t = p5.tile([128, E], f32, tag="x2r", bufs=2)
                            nc.sync.dma_start(
                                out=x2r_t,
                                in_=t["x2_buf"][b, sg * 128:(sg + 1) * 128, :])
                            out_t = p5.tile([128, E], f32, tag="out_t", bufs=2)
                            nc.vector.tensor_add(out_t, fn_t, x2r_t)
                            nc.sync.dma_start(
                                out=t["out"][b, sg * 128:(sg + 1) * 128, :],
                                in_=out_t)

        late_cm.__exit__(None, None, None)
    consts_cm.__exit__(None, None, None)


def prep_inputs(inputs, b_slice):
    """Host-side: build per-core in_map from full reference inputs."""
    x = np.ascontiguousarray(np.asarray(inputs["x"], np.float32)[b_slice])
    def wt(w):  # [H,E,D] -> [E, H*D]
        return np.ascontiguousarray(
            np.asarray(w, np.float32).transpose(1, 0, 2).reshape(E, E))
    m = {
        "x": x,
        "wq": wt(inputs["Wq"]).astype(ml_dtypes.bfloat16),
        "wk": wt(inputs["Wk"]).astype(ml_dtypes.bfloat16),
        "wv": wt(inputs["Wv"]).astype(ml_dtypes.bfloat16),
        "wp": np.ascontiguousarray(np.asarray(inputs["Wp"], np.float32)),
        "w1": np.ascontiguousarray(np.asarray(inputs["W1"], np.float32)),
        "w2": np.ascontiguousarray(np.asarray(inputs["W2"], np.float32)),
        "bq": np.asarray(inputs["bq"], np.float32).reshape(-1),
        "bk": np.asarray(inputs["bk"], np.float32).reshape(-1),
        "bv": np.asarray(inputs["bv"], np.float32).reshape(-1),
        "bp": np.asarray(inputs["bp"], np.float32).reshape(-1),
        "b1": np.asarray(inputs["b1"], np.float32).reshape(-1),
        "b2": np.asarray(inputs["b2"], np.float32).reshape(-1),
        "tri": np.triu(np.ones((128, 128), np.float32)),
        "sel2": np.concatenate([
            np.concatenate([np.ones((1, 64), np.float32),
                            np.zeros((1, 64), np.float32)], axis=1),
            np.concatenate([np.zeros((1, 64), np.float32),
                            np.ones((1, 64), np.float32)], axis=1)], axis=0),
    }
    for v in ["ln1_g", "ln1_b", "ln2_g", "ln2_b", "lnf_g", "lnf_b"]:
        m[v] = np.asarray(inputs[v], np.float32).reshape(-1)
    return m


def make_cfg(inputs):
    def nz(a):
        return not np.allclose(np.asarray(a), 0.0)
    def ntriv(g, bb):
        return (not np.allclose(np.asarray(g), 1.0)) or nz(bb)
    return {
        "ln1_aff": ntriv(inputs["ln1_g"], inputs["ln1_b"]),
        "ln2_aff": ntriv(inputs["ln2_g"], inputs["ln2_b"]),
        "lnf_aff": ntriv(inputs["lnf_g"], inputs["lnf_b"]),
        "add_bp": nz(inputs["bp"]),
        "add_b2": nz(inputs["b2"]),
    }


_CACHE = {}


def _get_compiled(cfg_key, cfg):
    if cfg_key in _CACHE:
        return _CACHE[cfg_key]
    nc = bacc.Bacc("TRN2", target_bir_lowering=False, debug=False)
    t = declare_io(nc, b_loc=B_LOC)
    with tile.TileContext(nc) as tc:
        build(nc, tc, t, cfg)
    nc.compile()
    _CACHE[cfg_key] = nc
    return nc


def _run(inputs, trace=False):
    inputs = {k: np.asarray(v) for k, v in inputs.items()}
    cfg = make_cfg(inputs)
    cfg["b_loc"] = B_LOC
    cfg_key = tuple(sorted(cfg.items()))
    nc = _get_compiled(cfg_key, cfg)
    n_cores = 8
    in_maps = [prep_inputs(inputs, slice(B_LOC * i, B_LOC * (i + 1)))
               for i in range(n_cores)]
    res = run_bass_kernel_spmd(nc, in_maps, list(range(n_cores)), trace=trace)
    out = np.concatenate([res.results[i]["out"] for i in range(n_cores)], axis=0)
    return out.astype(np.float32), res


def kernel(**inputs):
    out, _ = _run(inputs, trace=False)
    return out


def run_traced(**inputs):
    """Like kernel() but with NTFF profiling; returns (out, exec_time_ns)."""
    try:
        _install_ntff_hook()
    except Exception as e:
        print(f"ntff hook unavailable ({e}); running untraced")
        out, _ = _run(inputs, trace=False)
        return out, None
    out, res = _run(inputs, trace=True)
    return out, res.exec_time_ns


def _install_ntff_hook():
    import types
    import antenv
    if getattr(antenv, "axon_hooks", None) is not None:
        return
    mod = types.ModuleType("antenv.axon_hooks")
    _state = {"hook": None}
    mod.set_axon_ntff_profile_hook = lambda h: _state.__setitem__("hook", h)
    mod.get_axon_ntff_profile_hook = lambda: _state["hook"]
    sys.modules["antenv.axon_hooks"] = mod
    antenv.axon_hooks = mod
    from trn_agent_boot.trn_boot import _ntff_profile_via_ctypes
    hook = _ntff_profile_via_ctypes("/opt/axon/libaxon_pjrt.so")
    if hook is not None:
        mod.set_axon_ntff_profile_hook(hook)

